# revision 22
# baseline (speedup 1.0000x reference)
"""Trainium2 Bass kernel for nn_AnsweringHead (gumbel-top-k subset operator).

Self-contained: hardcodes shapes/sharding. 8-core data-parallel over the
128 = B*S rows; each core processes 16 rows x full vocab.

Key algorithmic transform (validated numerically): the reference's
log-domain loop
    s += log(max(1-onehot, eps)); onehot = softmax(s); khot += onehot
is run in linear "u-space":
    u' = (u - u^2) / (1 - sum(u^2));  khot += u'
which is algebraically identical and removes all exp/log from the
256-iteration loop.

Per-core SBUF layout: 16 rows x 30528 (padded vocab) as a (128, 3816)
tile; partition p holds row (p % 16), vocab segment (p // 16). All
layout packing/unpacking is done host-side so device DMAs are plain.
Padded vocab columns get b = -1e30 so they carry exactly zero weight
through exp/softmax and are never selected.
"""
import sys
import numpy as np

try:
    import concourse.bass as bass  # noqa
except Exception:
    sys.path.insert(0, "/opt/trn_rl_repo")

import concourse.bass as bass
import concourse.bacc as bacc
import concourse.mybir as mybir
import concourse.tile as tile
from concourse.bass_utils import run_bass_kernel_spmd

AF = mybir.ActivationFunctionType
ALU = mybir.AluOpType
DT = mybir.dt

# problem constants
B, S, V, D = 2, 64, 30522, 768
K = 256
N_CORES = 8
R = 16                 # rows per core
SEG = 8                # partitions per row
CH = 477               # column chunk (psum-bank friendly: 477 <= 512)
F = SEG * CH           # 3816 free columns per partition
VPAD = SEG * F         # 30528 padded vocab
NPE = 7                # khot chunks accumulated on PE (rest on DVE)
PE_COLS = NPE * CH     # 3339
TAIL = F - PE_COLS     # 477
K_ITER = K - 1         # loop iterations after u0
BISECT = 34
DCH = D // 128         # 6 contraction chunks

STAGE = 5
_cache = {}


def _consts():
    eye = np.eye(128, dtype=np.float32)
    gpos = np.zeros((128, R), np.float32)
    for p in range(128):
        gpos[p, p % R] = 1.0
    gt = np.zeros((R, 128), np.float32)
    for p in range(128):
        gt[p % R, p] = 1.0
    msum = np.zeros((128, SEG), np.float32)
    for p in range(128):
        msum[p, p // R] = 1.0
    return eye, gpos, gt, msum


def _pack(a):
    """(16, VPAD) row-major -> (128, F) device layout"""
    return np.ascontiguousarray(
        a.reshape(R, SEG, F).transpose(1, 0, 2).reshape(128, F))


def _unpack(a):
    """(128, F) device layout -> (16, VPAD)"""
    return a.reshape(SEG, R, F).transpose(1, 0, 2).reshape(R, VPAD)


def build_nc(k_iter=K_ITER, stage=5):
    nc = bacc.Bacc("TRN2", target_bir_lowering=False, debug=False,
                   num_devices=N_CORES)
    f32 = DT.float32

    hT = nc.dram_tensor("hT", [128, DCH * R], f32, kind="ExternalInput")
    gu = nc.dram_tensor("gu", [128, F], f32, kind="ExternalInput")
    Wv = nc.dram_tensor("Wv", [128, DCH * VPAD], f32, kind="ExternalInput")
    bv = nc.dram_tensor("bv", [128, F], f32, kind="ExternalInput")

    o_sel = nc.dram_tensor("sel", [128, F], f32, kind="ExternalOutput")
    o_y = nc.dram_tensor("yv", [128, F], f32, kind="ExternalOutput")
    o_lp = nc.dram_tensor("lp", [SEG, F], f32, kind="ExternalOutput")

    eye_np, gpos_np, gt_np, msum_np = _consts()
    eye_d = nc.inline_tensor(eye_np, name="c_eye")
    gpos_d = nc.inline_tensor(gpos_np, name="c_gpos")
    gt_d = nc.inline_tensor(gt_np, name="c_gt")
    gtn_d = nc.inline_tensor((-gt_np).copy(), name="c_gtn")
    msum_d = nc.inline_tensor(msum_np, name="c_msum")

    with tile.TileContext(nc) as tc:
        with tc.tile_pool(name="persist", bufs=1) as pp, \
             tc.tile_pool(name="tiny", bufs=1) as tp:
            # constants to SBUF
            eye_sb = pp.tile([128, 128], f32, tag="eye")
            nc.sync.dma_start(eye_sb[:], eye_d.ap())
            gpos_sb = pp.tile([128, R], f32, tag="gpos")
            nc.sync.dma_start(gpos_sb[:], gpos_d.ap())
            gt_sb = pp.tile([R, 128], f32, tag="gt")
            nc.sync.dma_start(gt_sb[:], gt_d.ap())
            gtn_sb = pp.tile([R, 128], f32, tag="gtn")
            nc.sync.dma_start(gtn_sb[:], gtn_d.ap())
            msum_sb = pp.tile([128, SEG], f32, tag="msum")
            nc.sync.dma_start(msum_sb[:], msum_d.ap())

            hT_sb = pp.tile([128, DCH, R], f32, tag="hT")
            nc.sync.dma_start(hT_sb[:].rearrange("p c r -> p (c r)"), hT.ap())

            logits = pp.tile([128, F], f32, tag="logits")
            brep = pp.tile([128, F], f32, tag="brep")
            nc.sync.dma_start(brep[:], bv.ap())

            # ---- phase 1: logits = hT.T @ W (stream W in 477-col tiles) ----
            with tc.tile_pool(name="wstream", bufs=3) as wp, \
                 tc.tile_pool(name="ps1", bufs=3, space="PSUM") as ps1:
                for vt in range(VPAD // CH):
                    wt = wp.tile([128, DCH, CH], f32, tag="wt")
                    nc.sync.dma_start(
                        wt[:],
                        Wv.ap().rearrange("p (c v) -> p c v", v=VPAD)
                        [:, :, vt * CH:(vt + 1) * CH])
                    pt = ps1.tile([R, CH], f32, tag="pt")
                    for d in range(DCH):
                        nc.tensor.matmul(pt[:], hT_sb[:, d, :], wt[:, d, :],
                                         start=(d == 0), stop=(d == DCH - 1))
                    bt = wp.tile([R, CH], f32, tag="bt")
                    nc.vector.tensor_copy(bt[:], pt[:])
                    k_vt, j_vt = vt // SEG, vt % SEG
                    nc.sync.dma_start(
                        logits[k_vt * R:(k_vt + 1) * R,
                               j_vt * CH:(j_vt + 1) * CH], bt[:])

            # b add (padded cols have b = -1e30 -> logits_pad = -1e30)
            nc.vector.tensor_tensor(logits[:], logits[:], brep[:], ALU.add)

            if stage < 2:
                nc.sync.dma_start(o_y.ap(), logits[:])
            else:
                _rest(nc, tc, pp, tp, stage, k_iter,
                      logits, eye_sb, gpos_sb, gt_sb, gtn_sb, msum_sb,
                      gu, o_sel, o_y, o_lp)
    nc.compile()
    return nc


def _rest(nc, tc, pp, tp, stage, k_iter,
          logits, eye_sb, gpos_sb, gt_sb, gtn_sb, msum_sb,
          gu, o_sel, o_y, o_lp):
    f32 = DT.float32
    # ---- phase 2: gumbel, e0, u0, and -LSE(logits) ----
    c1em20 = tp.tile([128, 1], f32, tag="c1em20")
    nc.vector.memset(c1em20[:], 1e-20)
    zp = tp.tile([128, 1], f32, tag="zp")
    sigp = tp.tile([128, 1], f32, tag="sigp")
    r16 = tp.tile([R, 1], f32, tag="r16")
    rbc = tp.tile([128, 1], f32, tag="rbc")
    nlse = tp.tile([128, 1], f32, tag="nlse")

    with tc.tile_pool(name="ph2", bufs=1) as p2, \
         tc.tile_pool(name="pst", bufs=1, space="PSUM") as pst:
        ps_t = pst.tile([128, 2], f32, tag="ps_t")

        # -LSE first (uses logits only)
        t_e = p2.tile([128, F], f32, tag="ph2a")
        nc.scalar.activation(t_e[:], logits[:], AF.Exp, accum_out=zp[:])
        nc.tensor.matmul(ps_t[0:R, 0:1], gpos_sb[:], zp[:],
                         start=True, stop=True, skip_group_check=True)
        nc.scalar.activation(r16[:], ps_t[0:R, 0:1], AF.Ln)
        nc.tensor.matmul(ps_t[:, 1:2], gtn_sb[:], r16[:],
                         start=True, stop=True, skip_group_check=True)
        nc.vector.tensor_copy(nlse[:], ps_t[:, 1:2])

        # gumbel -> scores -> e0 -> u0
        gsb = p2.tile([128, F], f32, tag="ph2a")
        nc.sync.dma_start(gsb[:], gu.ap())
        t1 = p2.tile([128, F], f32, tag="ph2b")
        nc.scalar.activation(t1[:], gsb[:], AF.Ln, bias=c1em20[:])
        t2 = p2.tile([128, F], f32, tag="ph2a")
        nc.scalar.activation(t2[:], t1[:], AF.Ln, bias=c1em20[:], scale=-1.0)
        scores = p2.tile([128, F], f32, tag="ph2b")
        nc.vector.tensor_tensor(scores[:], logits[:], t2[:], ALU.subtract)
        e0 = p2.tile([128, F], f32, tag="ph2a")
        nc.scalar.activation(e0[:], scores[:], AF.Exp, accum_out=zp[:])
        nc.tensor.matmul(ps_t[0:R, 0:1], gpos_sb[:], zp[:],
                         start=True, stop=True, skip_group_check=True)
        nc.vector.reciprocal(r16[:], ps_t[0:R, 0:1])
        nc.tensor.matmul(ps_t[:, 1:2], gt_sb[:], r16[:],
                         start=True, stop=True, skip_group_check=True)
        nc.vector.tensor_copy(rbc[:], ps_t[:, 1:2])

        u0 = pp.tile([128, F], f32, tag="u0")
        nc.vector.tensor_scalar(u0[:], e0[:], rbc[:], None, op0=ALU.mult)

        if stage < 3:
            nc.sync.dma_start(o_y.ap(), u0[:])
            return

        # ---- phase 3: the K-1 iteration loop ----
        khot_tail = pp.tile([128, TAIL], f32, tag="ktail")
        nc.vector.tensor_copy(khot_tail[:], u0[:, PE_COLS:])

        with tc.tile_pool(name="kps", bufs=1, space="PSUM") as kpsp, \
             tc.tile_pool(name="lp3", bufs=1) as p3:
            khot_ps = kpsp.tile([128, NPE, 512], f32, tag="khot")
            for j in range(NPE):
                nc.tensor.matmul(
                    khot_ps[:, j, 0:CH], eye_sb[:],
                    u0[:, j * CH:(j + 1) * CH],
                    start=True, stop=False, skip_group_check=True)

            u_cur = u0
            s16 = tp.tile([R, 1], f32, tag="s16")
            for t in range(k_iter):
                last = (t == k_iter - 1)
                q = p3.tile([128, F], f32, tag="q")
                nc.scalar.activation(q[:], u_cur[:], AF.Square,
                                     accum_out=sigp[:])
                nc.tensor.matmul(ps_t[0:R, 0:1], gpos_sb[:], sigp[:],
                                 start=True, stop=True, skip_group_check=True)
                nc.vector.tensor_scalar(s16[:], ps_t[0:R, 0:1], -1.0, 1.0,
                                        op0=ALU.mult, op1=ALU.add)
                nc.vector.reciprocal(r16[:], s16[:])
                nc.tensor.matmul(ps_t[:, 1:2], gt_sb[:], r16[:],
                                 start=True, stop=True, skip_group_check=True)
                nc.vector.tensor_copy(rbc[:], ps_t[:, 1:2])
                w = p3.tile([128, F], f32, tag="w")
                nc.vector.tensor_tensor(w[:], u_cur[:], q[:], ALU.subtract)
                u_nxt = p3.tile([128, F], f32, tag="un")
                nc.vector.tensor_scalar(u_nxt[:], w[:], rbc[:], None,
                                        op0=ALU.mult)
                for j in range(NPE):
                    nc.tensor.matmul(
                        khot_ps[:, j, 0:CH], eye_sb[:],
                        u_nxt[:, j * CH:(j + 1) * CH],
                        start=False, stop=last, skip_group_check=True)
                nc.vector.tensor_tensor(khot_tail[:], khot_tail[:],
                                        u_nxt[:, PE_COLS:], ALU.add)
                u_cur = u_nxt

            khot = pp.tile([128, F], f32, tag="khot_sb")
            nc.vector.tensor_copy(
                khot[:, 0:PE_COLS].rearrange("p (j c) -> p j c", c=CH),
                khot_ps[:, :, 0:CH])
            nc.vector.tensor_copy(khot[:, PE_COLS:], khot_tail[:])

        if stage < 4:
            nc.sync.dma_start(o_y.ap(), khot[:])
            return

        # ---- phase 4: top-K threshold bisection ----
        sel = pp.tile([128, F], f32, tag="sel")
        with tc.tile_pool(name="bis", bufs=4) as bp:
            lo = bp.tile([R, 1], f32, tag="lo")
            nc.vector.memset(lo[:], 0.0)
            hi = bp.tile([R, 1], f32, tag="hi")
            nc.vector.memset(hi[:], 8.0)
            for it in range(BISECT):
                mid = bp.tile([R, 1], f32, tag="mid")
                nc.vector.tensor_tensor(mid[:], lo[:], hi[:], ALU.add)
                nc.vector.tensor_scalar(mid[:], mid[:], 0.5, None,
                                        op0=ALU.mult)
                nc.tensor.matmul(ps_t[:, 1:2], gt_sb[:], mid[:],
                                 start=True, stop=True, skip_group_check=True)
                nc.vector.tensor_copy(rbc[:], ps_t[:, 1:2])
                nc.vector.tensor_scalar(sel[:], khot[:], rbc[:], None,
                                        op0=ALU.is_ge, op1=ALU.add,
                                        accum_out=sigp[:])
                nc.tensor.matmul(ps_t[0:R, 0:1], gpos_sb[:], sigp[:],
                                 start=True, stop=True, skip_group_check=True)
                c16 = bp.tile([R, 1], f32, tag="c16")
                nc.vector.tensor_copy(c16[:], ps_t[0:R, 0:1])
                m16 = bp.tile([R, 1], DT.int32, tag="m16")
                nc.vector.tensor_scalar(m16[:], c16[:], float(K), None,
                                        op0=ALU.is_ge)
                lo2 = bp.tile([R, 1], f32, tag="lo")
                nc.vector.select(lo2[:], m16[:], mid[:], lo[:])
                hi2 = bp.tile([R, 1], f32, tag="hi")
                nc.vector.select(hi2[:], m16[:], hi[:], mid[:])
                lo, hi = lo2, hi2
            nc.tensor.matmul(ps_t[:, 1:2], gt_sb[:], lo[:],
                             start=True, stop=True, skip_group_check=True)
            nc.vector.tensor_copy(rbc[:], ps_t[:, 1:2])
            nc.vector.tensor_scalar(sel[:], khot[:], rbc[:], None,
                                    op0=ALU.is_ge)

    if stage < 5:
        nc.sync.dma_start(o_sel.ap(), sel[:])
        return

    # ---- phase 5: epilogue ----
    with tc.tile_pool(name="ph5", bufs=1) as p5, \
         tc.tile_pool(name="ps5", bufs=1, space="PSUM") as ps5:
        lsm = p5.tile([128, F], f32, tag="ph5a")
        nc.vector.tensor_scalar(lsm[:], logits[:], nlse[:], None, op0=ALU.add)
        nc.vector.tensor_tensor(lsm[:], lsm[:], sel[:], ALU.mult)
        lp_ps = ps5.tile([SEG, SEG, 512], f32, tag="lp")
        for j in range(SEG):
            nc.tensor.matmul(lp_ps[:, j, 0:CH], msum_sb[:],
                             lsm[:, j * CH:(j + 1) * CH],
                             start=True, stop=True, skip_group_check=True)
        lp_sb = p5.tile([SEG, F], f32, tag="lpsb")
        nc.vector.tensor_copy(
            lp_sb[:].rearrange("p (j c) -> p j c", c=CH),
            lp_ps[:, :, 0:CH])
        nc.sync.dma_start(o_lp.ap(), lp_sb[:])

        yv = p5.tile([128, F], f32, tag="ph5b")
        nc.vector.tensor_tensor(yv[:], logits[:], sel[:], ALU.mult)
        nc.vector.tensor_scalar(yv[:], yv[:], 0.0, None, op0=ALU.max)
        nc.scalar.activation(yv[:], yv[:], AF.Ln, bias=1.0)
        nc.sync.dma_start(o_y.ap(), yv[:])
        nc.sync.dma_start(o_sel.ap(), sel[:])


def kernel(input_ids, attention_mask, gumbel_u, embed_table, W_vocab, b_vocab):
    input_ids = np.asarray(input_ids)
    attention_mask = np.asarray(attention_mask, np.float32)
    gumbel_u = np.asarray(gumbel_u, np.float32)
    embed_table = np.asarray(embed_table, np.float32)
    W_vocab = np.asarray(W_vocab, np.float32)
    b_vocab = np.asarray(b_vocab, np.float32)

    key = ("nc", K_ITER, STAGE)
    if key not in _cache:
        _cache[key] = build_nc(k_iter=K_ITER, stage=STAGE)
    nc = _cache[key]

    h = embed_table[input_ids.reshape(-1)]          # (128, D)
    Wp = np.zeros((D, VPAD), np.float32)
    Wp[:, :V] = W_vocab
    # (768, VPAD) -> (128, DCH*VPAD): partition p holds rows {c*128+p}
    Wa = np.ascontiguousarray(
        Wp.reshape(DCH, 128, VPAD).transpose(1, 0, 2).reshape(128, -1))
    bp = np.full((VPAD,), -1e30, np.float32)
    bp[:V] = b_vocab
    brep = _pack(np.tile(bp, (R, 1)))
    gup = np.full((B * S, VPAD), 0.5, np.float32)
    gup[:, :V] = gumbel_u

    in_maps = []
    for c in range(N_CORES):
        rows = slice(c * R, (c + 1) * R)
        hTa = np.ascontiguousarray(
            h[rows].T.reshape(DCH, 128, R).transpose(1, 0, 2).reshape(128, -1))
        in_maps.append({
            "hT": hTa,
            "gu": _pack(gup[rows]),
            "Wv": Wa,
            "bv": brep,
        })

    res = run_bass_kernel_spmd(nc, in_maps, core_ids=list(range(N_CORES)))
    if STAGE < 5:
        return res

    sel = np.concatenate(
        [_unpack(res.results[c]["sel"])[:, :V] for c in range(N_CORES)],
        axis=0).reshape(B, S, V)
    yv = np.concatenate(
        [_unpack(res.results[c]["yv"])[:, :V] for c in range(N_CORES)],
        axis=0).reshape(B, S, V)
    yv = yv * attention_mask.reshape(B, S, 1)
    values = yv.max(axis=1)
    lp = np.stack([res.results[c]["lp"].reshape(-1)[:V] for c in range(N_CORES)])
    logprobs = np.stack([lp[0:4].sum(axis=0), lp[4:8].sum(axis=0)])
    return (values.astype(np.float32), logprobs.astype(np.float32),
            sel.astype(np.float32))


# revision 24
# speedup vs baseline: 1.9869x; 1.9869x over previous
"""Trainium2 Bass kernel for nn_AnsweringHead (gumbel-top-k subset operator).

Self-contained: hardcodes shapes/sharding. 8-core data-parallel over the
128 = B*S rows; each core processes 16 rows x full vocab.

Key algorithmic transform (validated numerically): the reference's
log-domain loop
    s += log(max(1-onehot, eps)); onehot = softmax(s); khot += onehot
is run in linear "u-space":
    u' = (u - u^2) / (1 - sum(u^2));  khot += u'
which is algebraically identical and removes all exp/log from the
256-iteration loop.

Per-core SBUF layout: 16 rows x 30528 (padded vocab) as a (128, 3816)
tile; partition p holds row (p % 16), vocab segment (p // 16). All
layout packing/unpacking is done host-side so device DMAs are plain.
Padded vocab columns get b = -1e30 so they carry exactly zero weight
through exp/softmax and are never selected.
"""
import sys
import numpy as np
import ml_dtypes

try:
    import concourse.bass as bass  # noqa
except Exception:
    sys.path.insert(0, "/opt/trn_rl_repo")

import concourse.bass as bass
import concourse.bacc as bacc
import concourse.mybir as mybir
import concourse.tile as tile
from concourse.bass_utils import run_bass_kernel_spmd

AF = mybir.ActivationFunctionType
ALU = mybir.AluOpType
DT = mybir.dt

# problem constants
B, S, V, D = 2, 64, 30522, 768
K = 256
N_CORES = 8
R = 16                 # rows per core
SEG = 8                # partitions per row
CH = 477               # column chunk (psum-bank friendly: 477 <= 512)
F = SEG * CH           # 3816 free columns per partition
VPAD = SEG * F         # 30528 padded vocab
NPE = 7                # khot chunks accumulated on PE (rest on DVE)
PE_COLS = NPE * CH     # 3339
TAIL = F - PE_COLS     # 477
K_ITER = K - 1         # loop iterations after u0
BISECT = 24
DCH = D // 128         # 6 contraction chunks

STAGE = 5
_cache = {}


def _consts():
    eye = np.eye(128, dtype=np.float32)
    gpos = np.zeros((128, R), np.float32)
    for p in range(128):
        gpos[p, p % R] = 1.0
    gt = np.zeros((R, 128), np.float32)
    for p in range(128):
        gt[p % R, p] = 1.0
    msum = np.zeros((128, SEG), np.float32)
    for p in range(128):
        msum[p, p // R] = 1.0
    return eye, gpos, gt, msum


def _pack(a):
    """(16, VPAD) row-major -> (128, F) device layout"""
    return np.ascontiguousarray(
        a.reshape(R, SEG, F).transpose(1, 0, 2).reshape(128, F))


def _unpack(a):
    """(128, F) device layout -> (16, VPAD)"""
    return a.reshape(SEG, R, F).transpose(1, 0, 2).reshape(R, VPAD)


def build_nc(k_iter=K_ITER, stage=5):
    nc = bacc.Bacc("TRN2", target_bir_lowering=False, debug=False,
                   num_devices=N_CORES)
    f32 = DT.float32

    bf16 = DT.bfloat16
    hT = nc.dram_tensor("hT", [128, DCH * R], bf16, kind="ExternalInput")
    gu = nc.dram_tensor("gu", [128, F], f32, kind="ExternalInput")
    Wv = nc.dram_tensor("Wv", [128, DCH * VPAD], bf16, kind="ExternalInput")
    bv = nc.dram_tensor("bv", [128, F], f32, kind="ExternalInput")

    o_sel = nc.dram_tensor("sel", [128, F], f32, kind="ExternalOutput")
    o_y = nc.dram_tensor("yv", [128, F], f32, kind="ExternalOutput")
    o_lp = nc.dram_tensor("lp", [SEG, F], f32, kind="ExternalOutput")

    eye_np, gpos_np, gt_np, msum_np = _consts()
    eye_d = nc.inline_tensor(eye_np, name="c_eye")
    gpos_d = nc.inline_tensor(gpos_np, name="c_gpos")
    gt_d = nc.inline_tensor(gt_np, name="c_gt")
    gtn_d = nc.inline_tensor((-gt_np).copy(), name="c_gtn")
    msum_d = nc.inline_tensor(msum_np, name="c_msum")

    with tile.TileContext(nc) as tc:
        with tc.tile_pool(name="persist", bufs=1) as pp, \
             tc.tile_pool(name="tiny", bufs=1) as tp:
            # constants to SBUF
            eye_sb = pp.tile([128, 128], f32, tag="eye")
            nc.sync.dma_start(eye_sb[:], eye_d.ap())
            gpos_sb = pp.tile([128, R], f32, tag="gpos")
            nc.sync.dma_start(gpos_sb[:], gpos_d.ap())
            gt_sb = pp.tile([R, 128], f32, tag="gt")
            nc.sync.dma_start(gt_sb[:], gt_d.ap())
            gtn_sb = pp.tile([R, 128], f32, tag="gtn")
            nc.sync.dma_start(gtn_sb[:], gtn_d.ap())
            msum_sb = pp.tile([128, SEG], f32, tag="msum")
            nc.sync.dma_start(msum_sb[:], msum_d.ap())

            hT_sb = pp.tile([128, DCH, R], DT.bfloat16, tag="hT")
            nc.sync.dma_start(hT_sb[:].rearrange("p c r -> p (c r)"), hT.ap())

            logits = pp.tile([128, F], f32, tag="logits")
            brep = pp.tile([128, F], f32, tag="brep")
            nc.sync.dma_start(brep[:], bv.ap())

            # ---- phase 1: logits = hT.T @ W (stream W in 477-col tiles) ----
            with tc.tile_pool(name="wstream", bufs=3) as wp, \
                 tc.tile_pool(name="ps1", bufs=3, space="PSUM") as ps1:
                for vt in range(VPAD // CH):
                    wt = wp.tile([128, DCH, CH], DT.bfloat16, tag="wt")
                    nc.sync.dma_start(
                        wt[:],
                        Wv.ap().rearrange("p (c v) -> p c v", v=VPAD)
                        [:, :, vt * CH:(vt + 1) * CH])
                    pt = ps1.tile([R, CH], f32, tag="pt")
                    for d in range(DCH):
                        nc.tensor.matmul(pt[:], hT_sb[:, d, :], wt[:, d, :],
                                         start=(d == 0), stop=(d == DCH - 1))
                    bt = wp.tile([R, CH], f32, tag="bt")
                    nc.vector.tensor_copy(bt[:], pt[:])
                    k_vt, j_vt = vt // SEG, vt % SEG
                    nc.sync.dma_start(
                        logits[k_vt * R:(k_vt + 1) * R,
                               j_vt * CH:(j_vt + 1) * CH], bt[:])

            # b add (padded cols have b = -1e30 -> logits_pad = -1e30)
            nc.vector.tensor_tensor(logits[:], logits[:], brep[:], ALU.add)

            if stage < 2:
                nc.sync.dma_start(o_y.ap(), logits[:])
            else:
                _rest(nc, tc, pp, tp, stage, k_iter,
                      logits, eye_sb, gpos_sb, gt_sb, gtn_sb, msum_sb,
                      gu, o_sel, o_y, o_lp)
    nc.compile()
    return nc


def _rest(nc, tc, pp, tp, stage, k_iter,
          logits, eye_sb, gpos_sb, gt_sb, gtn_sb, msum_sb,
          gu, o_sel, o_y, o_lp):
    f32 = DT.float32
    # ---- phase 2: gumbel, e0, u0, and -LSE(logits) ----
    c1em20 = tp.tile([128, 1], f32, tag="c1em20")
    nc.vector.memset(c1em20[:], 1e-20)
    zp = tp.tile([128, 1], f32, tag="zp")
    sigp = tp.tile([128, 1], f32, tag="sigp")
    r16 = tp.tile([R, 1], f32, tag="r16")
    rbc = tp.tile([128, 1], f32, tag="rbc")
    nlse = tp.tile([128, 1], f32, tag="nlse")

    with tc.tile_pool(name="ph2", bufs=1) as p2, \
         tc.tile_pool(name="pst", bufs=1, space="PSUM") as pst:
        ps_t = pst.tile([128, 2], f32, tag="ps_t")

        # -LSE first (uses logits only)
        t_e = p2.tile([128, F], f32, tag="ph2a")
        nc.scalar.activation(t_e[:], logits[:], AF.Exp, accum_out=zp[:])
        nc.tensor.matmul(ps_t[0:R, 0:1], gpos_sb[:], zp[:],
                         start=True, stop=True, skip_group_check=True)
        nc.scalar.activation(r16[:], ps_t[0:R, 0:1], AF.Ln)
        nc.tensor.matmul(ps_t[:, 1:2], gtn_sb[:], r16[:],
                         start=True, stop=True, skip_group_check=True)
        nc.vector.tensor_copy(nlse[:], ps_t[:, 1:2])

        # gumbel -> scores -> e0 -> u0
        gsb = p2.tile([128, F], f32, tag="ph2a")
        nc.sync.dma_start(gsb[:], gu.ap())
        t1 = p2.tile([128, F], f32, tag="ph2b")
        nc.scalar.activation(t1[:], gsb[:], AF.Ln, bias=c1em20[:])
        t2 = p2.tile([128, F], f32, tag="ph2a")
        nc.scalar.activation(t2[:], t1[:], AF.Ln, bias=c1em20[:], scale=-1.0)
        scores = p2.tile([128, F], f32, tag="ph2b")
        nc.vector.tensor_tensor(scores[:], logits[:], t2[:], ALU.subtract)
        e0 = p2.tile([128, F], f32, tag="ph2a")
        nc.scalar.activation(e0[:], scores[:], AF.Exp, accum_out=zp[:])
        nc.tensor.matmul(ps_t[0:R, 0:1], gpos_sb[:], zp[:],
                         start=True, stop=True, skip_group_check=True)
        nc.vector.reciprocal(r16[:], ps_t[0:R, 0:1])
        nc.tensor.matmul(ps_t[:, 1:2], gt_sb[:], r16[:],
                         start=True, stop=True, skip_group_check=True)
        nc.vector.tensor_copy(rbc[:], ps_t[:, 1:2])

        u0 = pp.tile([128, F], f32, tag="u0")
        nc.vector.tensor_scalar(u0[:], e0[:], rbc[:], None, op0=ALU.mult)

        if stage < 3:
            nc.sync.dma_start(o_y.ap(), u0[:])
            return

        # ---- phase 3: the K-1 iteration loop ----
        khot_tail = pp.tile([128, TAIL], f32, tag="ktail")
        nc.vector.tensor_copy(khot_tail[:], u0[:, PE_COLS:])

        with tc.tile_pool(name="kps", bufs=1, space="PSUM") as kpsp, \
             tc.tile_pool(name="lp3", bufs=2) as p3:
            khot_ps = kpsp.tile([128, NPE, 512], f32, tag="khot")
            for j in range(NPE):
                nc.tensor.matmul(
                    khot_ps[:, j, 0:CH], eye_sb[:],
                    u0[:, j * CH:(j + 1) * CH],
                    start=True, stop=False, skip_group_check=True)

            u_cur = u0
            HF = F // 2
            cnh = tp.tile([128, 1], f32, tag="cnh")
            nc.vector.memset(cnh[:], -0.5)
            sig2 = tp.tile([128, 2], f32, tag="sig2")
            for t in range(k_iter):
                last = (t == k_iter - 1)
                # p = (u - 0.5)^2, two chunks for pipelining
                q = p3.tile([128, F], f32, tag="q")
                nc.scalar.activation(q[:, 0:HF], u_cur[:, 0:HF], AF.Square,
                                     bias=cnh[:])
                nc.scalar.activation(q[:, HF:F], u_cur[:, HF:F], AF.Square,
                                     bias=cnh[:])
                # wt = p - 0.25 (= u^2 - u); accum -> -sigma partials
                w = p3.tile([128, F], f32, tag="w")
                nc.vector.tensor_scalar(w[:, 0:HF], q[:, 0:HF], 0.25, None,
                                        op0=ALU.subtract, op1=ALU.add,
                                        accum_out=sig2[:, 0:1])
                nc.vector.tensor_scalar(w[:, HF:F], q[:, HF:F], 0.25, None,
                                        op0=ALU.subtract, op1=ALU.add,
                                        accum_out=sig2[:, 1:2])
                nc.tensor.matmul(ps_t[0:R, 0:1], gpos_sb[:], sig2[:, 0:1],
                                 start=True, stop=False, skip_group_check=True)
                nc.tensor.matmul(ps_t[0:R, 0:1], gpos_sb[:], sig2[:, 1:2],
                                 start=False, stop=True, skip_group_check=True)
                # -r = 1/(-sigma)
                nc.vector.reciprocal(r16[:], ps_t[0:R, 0:1])
                nc.tensor.matmul(ps_t[:, 1:2], gt_sb[:], r16[:],
                                 start=True, stop=True, skip_group_check=True)
                nc.vector.tensor_copy(rbc[:], ps_t[:, 1:2])
                # u' = wt * (-r)
                u_nxt = p3.tile([128, F], f32, tag="un")
                nc.vector.tensor_scalar(u_nxt[:], w[:], rbc[:], None,
                                        op0=ALU.mult)
                for j in range(NPE):
                    nc.tensor.matmul(
                        khot_ps[:, j, 0:CH], eye_sb[:],
                        u_nxt[:, j * CH:(j + 1) * CH],
                        start=False, stop=last, skip_group_check=True)
                nc.vector.tensor_tensor(khot_tail[:], khot_tail[:],
                                        u_nxt[:, PE_COLS:], ALU.add)
                u_cur = u_nxt

            khot = pp.tile([128, F], f32, tag="khot_sb")
            nc.vector.tensor_copy(
                khot[:, 0:PE_COLS].rearrange("p (j c) -> p j c", c=CH),
                khot_ps[:, :, 0:CH])
            nc.vector.tensor_copy(khot[:, PE_COLS:], khot_tail[:])

        if stage < 4:
            nc.sync.dma_start(o_y.ap(), khot[:])
            return

        # ---- phase 4: top-K threshold bisection ----
        sel = pp.tile([128, F], f32, tag="sel")
        with tc.tile_pool(name="bis", bufs=4) as bp:
            lo = bp.tile([R, 1], f32, tag="lo")
            nc.vector.memset(lo[:], 0.0)
            hi = bp.tile([R, 1], f32, tag="hi")
            nc.vector.memset(hi[:], 8.0)
            for it in range(BISECT):
                mid = bp.tile([R, 1], f32, tag="mid")
                nc.vector.tensor_tensor(mid[:], lo[:], hi[:], ALU.add)
                nc.vector.tensor_scalar(mid[:], mid[:], 0.5, None,
                                        op0=ALU.mult)
                nc.tensor.matmul(ps_t[:, 1:2], gt_sb[:], mid[:],
                                 start=True, stop=True, skip_group_check=True)
                nc.vector.tensor_copy(rbc[:], ps_t[:, 1:2])
                nc.vector.tensor_scalar(sel[:], khot[:], rbc[:], None,
                                        op0=ALU.is_ge, op1=ALU.add,
                                        accum_out=sigp[:])
                nc.tensor.matmul(ps_t[0:R, 0:1], gpos_sb[:], sigp[:],
                                 start=True, stop=True, skip_group_check=True)
                c16 = bp.tile([R, 1], f32, tag="c16")
                nc.vector.tensor_copy(c16[:], ps_t[0:R, 0:1])
                m16 = bp.tile([R, 1], DT.int32, tag="m16")
                nc.vector.tensor_scalar(m16[:], c16[:], float(K), None,
                                        op0=ALU.is_ge)
                lo2 = bp.tile([R, 1], f32, tag="lo")
                nc.vector.select(lo2[:], m16[:], mid[:], lo[:])
                hi2 = bp.tile([R, 1], f32, tag="hi")
                nc.vector.select(hi2[:], m16[:], hi[:], mid[:])
                lo, hi = lo2, hi2
            nc.tensor.matmul(ps_t[:, 1:2], gt_sb[:], lo[:],
                             start=True, stop=True, skip_group_check=True)
            nc.vector.tensor_copy(rbc[:], ps_t[:, 1:2])
            nc.vector.tensor_scalar(sel[:], khot[:], rbc[:], None,
                                    op0=ALU.is_ge)

    if stage < 5:
        nc.sync.dma_start(o_sel.ap(), sel[:])
        return

    # ---- phase 5: epilogue ----
    with tc.tile_pool(name="ph5", bufs=1) as p5, \
         tc.tile_pool(name="ps5", bufs=1, space="PSUM") as ps5:
        lsm = p5.tile([128, F], f32, tag="ph5a")
        nc.vector.tensor_scalar(lsm[:], logits[:], nlse[:], None, op0=ALU.add)
        nc.vector.tensor_tensor(lsm[:], lsm[:], sel[:], ALU.mult)
        lp_ps = ps5.tile([SEG, SEG, 512], f32, tag="lp")
        for j in range(SEG):
            nc.tensor.matmul(lp_ps[:, j, 0:CH], msum_sb[:],
                             lsm[:, j * CH:(j + 1) * CH],
                             start=True, stop=True, skip_group_check=True)
        lp_sb = p5.tile([SEG, F], f32, tag="lpsb")
        nc.vector.tensor_copy(
            lp_sb[:].rearrange("p (j c) -> p j c", c=CH),
            lp_ps[:, :, 0:CH])
        nc.sync.dma_start(o_lp.ap(), lp_sb[:])

        yv = p5.tile([128, F], f32, tag="ph5b")
        nc.vector.tensor_tensor(yv[:], logits[:], sel[:], ALU.mult)
        nc.vector.tensor_scalar(yv[:], yv[:], 0.0, None, op0=ALU.max)
        nc.scalar.activation(yv[:], yv[:], AF.Ln, bias=1.0)
        nc.sync.dma_start(o_y.ap(), yv[:])
        nc.sync.dma_start(o_sel.ap(), sel[:])


def kernel(input_ids, attention_mask, gumbel_u, embed_table, W_vocab, b_vocab):
    input_ids = np.asarray(input_ids)
    attention_mask = np.asarray(attention_mask, np.float32)
    gumbel_u = np.asarray(gumbel_u, np.float32)
    embed_table = np.asarray(embed_table, np.float32)
    W_vocab = np.asarray(W_vocab, np.float32)
    b_vocab = np.asarray(b_vocab, np.float32)

    key = ("nc", K_ITER, STAGE)
    if key not in _cache:
        _cache[key] = build_nc(k_iter=K_ITER, stage=STAGE)
    nc = _cache[key]

    h = embed_table[input_ids.reshape(-1)]          # (128, D)
    Wp = np.zeros((D, VPAD), np.float32)
    Wp[:, :V] = W_vocab
    # (768, VPAD) -> (128, DCH*VPAD): partition p holds rows {c*128+p}
    Wa = np.ascontiguousarray(
        Wp.reshape(DCH, 128, VPAD).transpose(1, 0, 2).reshape(128, -1)
        .astype(ml_dtypes.bfloat16))
    bp = np.full((VPAD,), -1e30, np.float32)
    bp[:V] = b_vocab
    brep = _pack(np.tile(bp, (R, 1)))
    gup = np.full((B * S, VPAD), 0.5, np.float32)
    gup[:, :V] = gumbel_u

    in_maps = []
    for c in range(N_CORES):
        rows = slice(c * R, (c + 1) * R)
        hTa = np.ascontiguousarray(
            h[rows].T.reshape(DCH, 128, R).transpose(1, 0, 2).reshape(128, -1)
            .astype(ml_dtypes.bfloat16))
        in_maps.append({
            "hT": hTa,
            "gu": _pack(gup[rows]),
            "Wv": Wa,
            "bv": brep,
        })

    res = run_bass_kernel_spmd(nc, in_maps, core_ids=list(range(N_CORES)))
    if STAGE < 5:
        return res

    sel = np.concatenate(
        [_unpack(res.results[c]["sel"])[:, :V] for c in range(N_CORES)],
        axis=0).reshape(B, S, V)
    yv = np.concatenate(
        [_unpack(res.results[c]["yv"])[:, :V] for c in range(N_CORES)],
        axis=0).reshape(B, S, V)
    yv = yv * attention_mask.reshape(B, S, 1)
    values = yv.max(axis=1)
    lp = np.stack([res.results[c]["lp"].reshape(-1)[:V] for c in range(N_CORES)])
    logprobs = np.stack([lp[0:4].sum(axis=0), lp[4:8].sum(axis=0)])
    return (values.astype(np.float32), logprobs.astype(np.float32),
            sel.astype(np.float32))


# revision 26
# speedup vs baseline: 2.0772x; 1.0455x over previous
"""Trainium2 Bass kernel for nn_AnsweringHead (gumbel-top-k subset operator).

Self-contained: hardcodes shapes/sharding. 8-core data-parallel over the
128 = B*S rows; each core processes 16 rows x full vocab.

Key algorithmic transform (validated numerically): the reference's
log-domain loop
    s += log(max(1-onehot, eps)); onehot = softmax(s); khot += onehot
is run in linear "u-space":
    u' = (u - u^2) / (1 - sum(u^2));  khot += u'
which is algebraically identical and removes all exp/log from the
256-iteration loop.

Per-core SBUF layout: 16 rows x 30528 (padded vocab) as a (128, 3816)
tile; partition p holds row (p % 16), vocab segment (p // 16). All
layout packing/unpacking is done host-side so device DMAs are plain.
Padded vocab columns get b = -1e30 so they carry exactly zero weight
through exp/softmax and are never selected.
"""
import sys
import numpy as np
import ml_dtypes

try:
    import concourse.bass as bass  # noqa
except Exception:
    sys.path.insert(0, "/opt/trn_rl_repo")

import concourse.bass as bass
import concourse.bacc as bacc
import concourse.mybir as mybir
import concourse.tile as tile
from concourse.bass_utils import run_bass_kernel_spmd

AF = mybir.ActivationFunctionType
ALU = mybir.AluOpType
DT = mybir.dt

# problem constants
B, S, V, D = 2, 64, 30522, 768
K = 256
N_CORES = 8
R = 16                 # rows per core
SEG = 8                # partitions per row
CH = 477               # column chunk (psum-bank friendly: 477 <= 512)
F = SEG * CH           # 3816 free columns per partition
VPAD = SEG * F         # 30528 padded vocab
NPE = 7                # khot chunks accumulated on PE (rest on DVE)
PE_COLS = NPE * CH     # 3339
TAIL = F - PE_COLS     # 477
K_ITER = K - 1         # loop iterations after u0
BISECT = 24
DCH = D // 128         # 6 contraction chunks

STAGE = 5
_cache = {}


def _consts():
    eye = np.eye(128, dtype=np.float32)
    gpos = np.zeros((128, R), np.float32)
    for p in range(128):
        gpos[p, p % R] = 1.0
    gt = np.zeros((R, 128), np.float32)
    for p in range(128):
        gt[p % R, p] = 1.0
    msum = np.zeros((128, SEG), np.float32)
    for p in range(128):
        msum[p, p // R] = 1.0
    return eye, gpos, gt, msum


def _pack(a):
    """(16, VPAD) row-major -> (128, F) device layout"""
    return np.ascontiguousarray(
        a.reshape(R, SEG, F).transpose(1, 0, 2).reshape(128, F))


def _unpack(a):
    """(128, F) device layout -> (16, VPAD)"""
    return a.reshape(SEG, R, F).transpose(1, 0, 2).reshape(R, VPAD)


def build_nc(k_iter=K_ITER, stage=5):
    nc = bacc.Bacc("TRN2", target_bir_lowering=False, debug=False,
                   num_devices=N_CORES)
    f32 = DT.float32

    bf16 = DT.bfloat16
    hT = nc.dram_tensor("hT", [128, DCH * R], bf16, kind="ExternalInput")
    gu = nc.dram_tensor("gu", [128, F], f32, kind="ExternalInput")
    Wv = nc.dram_tensor("Wv", [128, DCH * VPAD], bf16, kind="ExternalInput")
    bv = nc.dram_tensor("bv", [128, F], f32, kind="ExternalInput")

    o_sel = nc.dram_tensor("sel", [128, F], f32, kind="ExternalOutput")
    o_y = nc.dram_tensor("yv", [128, F], f32, kind="ExternalOutput")
    o_lp = nc.dram_tensor("lp", [SEG, F], f32, kind="ExternalOutput")

    eye_np, gpos_np, gt_np, msum_np = _consts()
    eye_d = nc.inline_tensor(eye_np, name="c_eye")
    gpos_d = nc.inline_tensor(gpos_np, name="c_gpos")
    gt_d = nc.inline_tensor(gt_np, name="c_gt")
    gtn_d = nc.inline_tensor((-gt_np).copy(), name="c_gtn")
    msum_d = nc.inline_tensor(msum_np, name="c_msum")

    with tile.TileContext(nc) as tc:
        with tc.tile_pool(name="persist", bufs=1) as pp, \
             tc.tile_pool(name="tiny", bufs=1) as tp:
            # constants to SBUF
            eye_sb = pp.tile([128, 128], f32, tag="eye")
            nc.sync.dma_start(eye_sb[:], eye_d.ap())
            gpos_sb = pp.tile([128, R], f32, tag="gpos")
            nc.sync.dma_start(gpos_sb[:], gpos_d.ap())
            gt_sb = pp.tile([R, 128], f32, tag="gt")
            nc.sync.dma_start(gt_sb[:], gt_d.ap())
            gtn_sb = pp.tile([R, 128], f32, tag="gtn")
            nc.sync.dma_start(gtn_sb[:], gtn_d.ap())
            msum_sb = pp.tile([128, SEG], f32, tag="msum")
            nc.sync.dma_start(msum_sb[:], msum_d.ap())

            hT_sb = pp.tile([128, DCH, R], DT.bfloat16, tag="hT")
            nc.sync.dma_start(hT_sb[:].rearrange("p c r -> p (c r)"), hT.ap())

            logits = pp.tile([128, F], f32, tag="logits")
            brep = pp.tile([128, F], f32, tag="brep")
            nc.sync.dma_start(brep[:], bv.ap())

            # ---- phase 1: logits = hT.T @ W (stream W in 477-col tiles) ----
            with tc.tile_pool(name="wstream", bufs=3) as wp, \
                 tc.tile_pool(name="ps1", bufs=3, space="PSUM") as ps1:
                for vt in range(VPAD // CH):
                    wt = wp.tile([128, DCH, CH], DT.bfloat16, tag="wt")
                    nc.sync.dma_start(
                        wt[:],
                        Wv.ap().rearrange("p (c v) -> p c v", v=VPAD)
                        [:, :, vt * CH:(vt + 1) * CH])
                    pt = ps1.tile([R, CH], f32, tag="pt")
                    for d in range(DCH):
                        nc.tensor.matmul(pt[:], hT_sb[:, d, :], wt[:, d, :],
                                         start=(d == 0), stop=(d == DCH - 1))
                    bt = wp.tile([R, CH], f32, tag="bt")
                    nc.vector.tensor_copy(bt[:], pt[:])
                    k_vt, j_vt = vt // SEG, vt % SEG
                    nc.sync.dma_start(
                        logits[k_vt * R:(k_vt + 1) * R,
                               j_vt * CH:(j_vt + 1) * CH], bt[:])

            # b add (padded cols have b = -1e30 -> logits_pad = -1e30)
            nc.vector.tensor_tensor(logits[:], logits[:], brep[:], ALU.add)

            if stage < 2:
                nc.sync.dma_start(o_y.ap(), logits[:])
            else:
                _rest(nc, tc, pp, tp, stage, k_iter,
                      logits, eye_sb, gpos_sb, gt_sb, gtn_sb, msum_sb,
                      gu, o_sel, o_y, o_lp)
    nc.compile()
    return nc


def _rest(nc, tc, pp, tp, stage, k_iter,
          logits, eye_sb, gpos_sb, gt_sb, gtn_sb, msum_sb,
          gu, o_sel, o_y, o_lp):
    f32 = DT.float32
    # ---- phase 2: gumbel, e0, u0, and -LSE(logits) ----
    c1em20 = tp.tile([128, 1], f32, tag="c1em20")
    nc.vector.memset(c1em20[:], 1e-20)
    zp = tp.tile([128, 1], f32, tag="zp")
    sigp = tp.tile([128, 1], f32, tag="sigp")
    r16 = tp.tile([R, 1], f32, tag="r16")
    rbc = tp.tile([128, 1], f32, tag="rbc")
    nlse = tp.tile([128, 1], f32, tag="nlse")

    with tc.tile_pool(name="ph2", bufs=1) as p2, \
         tc.tile_pool(name="pst", bufs=1, space="PSUM") as pst:
        ps_t = pst.tile([128, 2], f32, tag="ps_t")

        # -LSE first (uses logits only)
        t_e = p2.tile([128, F], f32, tag="ph2a")
        nc.scalar.activation(t_e[:], logits[:], AF.Exp, accum_out=zp[:])
        nc.tensor.matmul(ps_t[0:R, 0:1], gpos_sb[:], zp[:],
                         start=True, stop=True, skip_group_check=True)
        nc.scalar.activation(r16[:], ps_t[0:R, 0:1], AF.Ln)
        nc.tensor.matmul(ps_t[:, 1:2], gtn_sb[:], r16[:],
                         start=True, stop=True, skip_group_check=True)
        nc.vector.tensor_copy(nlse[:], ps_t[:, 1:2])

        # gumbel -> scores -> e0 -> u0
        gsb = p2.tile([128, F], f32, tag="ph2a")
        nc.sync.dma_start(gsb[:], gu.ap())
        t1 = p2.tile([128, F], f32, tag="ph2b")
        nc.scalar.activation(t1[:], gsb[:], AF.Ln, bias=c1em20[:])
        t2 = p2.tile([128, F], f32, tag="ph2a")
        nc.scalar.activation(t2[:], t1[:], AF.Ln, bias=c1em20[:], scale=-1.0)
        scores = p2.tile([128, F], f32, tag="ph2b")
        nc.vector.tensor_tensor(scores[:], logits[:], t2[:], ALU.subtract)
        e0 = p2.tile([128, F], f32, tag="ph2a")
        nc.scalar.activation(e0[:], scores[:], AF.Exp, accum_out=zp[:])
        nc.tensor.matmul(ps_t[0:R, 0:1], gpos_sb[:], zp[:],
                         start=True, stop=True, skip_group_check=True)
        nc.vector.reciprocal(r16[:], ps_t[0:R, 0:1])
        nc.tensor.matmul(ps_t[:, 1:2], gt_sb[:], r16[:],
                         start=True, stop=True, skip_group_check=True)
        nc.vector.tensor_copy(rbc[:], ps_t[:, 1:2])

        u0 = pp.tile([128, F], f32, tag="u0")
        nc.vector.tensor_scalar(u0[:], e0[:], rbc[:], None, op0=ALU.mult)

        if stage < 3:
            nc.sync.dma_start(o_y.ap(), u0[:])
            return

        # ---- phase 3: the K-1 iteration loop ----
        khot = pp.tile([128, F], f32, tag="khot_sb")
        nc.vector.tensor_copy(khot[:], u0[:])

        with tc.tile_pool(name="lp3", bufs=2) as p3:
            u_cur = u0
            HF = F // 2
            cnh = tp.tile([128, 1], f32, tag="cnh")
            nc.vector.memset(cnh[:], -0.5)
            sig2 = tp.tile([128, 2], f32, tag="sig2")
            wt_prev = None
            for t in range(k_iter):
                last = (t == k_iter - 1)
                # p = (u - 0.5)^2 = (nrbc*wt_prev - 0.5)^2, chunked
                q = p3.tile([128, F], f32, tag="q")
                if t == 0:
                    nc.scalar.activation(q[:, 0:HF], u0[:, 0:HF], AF.Square,
                                         bias=cnh[:])
                    nc.scalar.activation(q[:, HF:F], u0[:, HF:F], AF.Square,
                                         bias=cnh[:])
                else:
                    nc.scalar.activation(q[:, 0:HF], wt_prev[:, 0:HF],
                                         AF.Square, bias=cnh[:],
                                         scale=rbc[:])
                    nc.scalar.activation(q[:, HF:F], wt_prev[:, HF:F],
                                         AF.Square, bias=cnh[:],
                                         scale=rbc[:])
                # wt = p - 0.25 (= u^2 - u); accum -> -sigma partials
                w = p3.tile([128, F], f32, tag="w")
                nc.vector.tensor_scalar(w[:, 0:HF], q[:, 0:HF], 0.25, None,
                                        op0=ALU.subtract, op1=ALU.add,
                                        accum_out=sig2[:, 0:1])
                nc.vector.tensor_scalar(w[:, HF:F], q[:, HF:F], 0.25, None,
                                        op0=ALU.subtract, op1=ALU.add,
                                        accum_out=sig2[:, 1:2])
                nc.tensor.matmul(ps_t[0:R, 0:1], gpos_sb[:], sig2[:, 0:1],
                                 start=True, stop=False, skip_group_check=True)
                nc.tensor.matmul(ps_t[0:R, 0:1], gpos_sb[:], sig2[:, 1:2],
                                 start=False, stop=True, skip_group_check=True)
                # -r = 1/(-sigma)
                nc.vector.reciprocal(r16[:], ps_t[0:R, 0:1])
                nc.tensor.matmul(ps_t[:, 1:2], gt_sb[:], r16[:],
                                 start=True, stop=True, skip_group_check=True)
                nc.vector.tensor_copy(rbc[:], ps_t[:, 1:2])
                # u' = wt * (-r)  (for khot accumulation only)
                u_nxt = p3.tile([128, F], f32, tag="un")
                nc.vector.tensor_scalar(u_nxt[:], w[:], rbc[:], None,
                                        op0=ALU.mult)
                wt_prev = w
                nc.vector.tensor_tensor(khot[:], khot[:], u_nxt[:], ALU.add)
                u_cur = u_nxt

        if stage < 4:
            nc.sync.dma_start(o_y.ap(), khot[:])
            return

        # ---- phase 4: top-K threshold bisection ----
        sel = pp.tile([128, F], f32, tag="sel")
        with tc.tile_pool(name="bis", bufs=4) as bp:
            lo = bp.tile([R, 1], f32, tag="lo")
            nc.vector.memset(lo[:], 0.0)
            hi = bp.tile([R, 1], f32, tag="hi")
            nc.vector.memset(hi[:], 8.0)
            for it in range(BISECT):
                mid = bp.tile([R, 1], f32, tag="mid")
                nc.vector.tensor_tensor(mid[:], lo[:], hi[:], ALU.add)
                nc.vector.tensor_scalar(mid[:], mid[:], 0.5, None,
                                        op0=ALU.mult)
                nc.tensor.matmul(ps_t[:, 1:2], gt_sb[:], mid[:],
                                 start=True, stop=True, skip_group_check=True)
                nc.vector.tensor_copy(rbc[:], ps_t[:, 1:2])
                nc.vector.tensor_scalar(sel[:], khot[:], rbc[:], None,
                                        op0=ALU.is_ge, op1=ALU.add,
                                        accum_out=sigp[:])
                nc.tensor.matmul(ps_t[0:R, 0:1], gpos_sb[:], sigp[:],
                                 start=True, stop=True, skip_group_check=True)
                c16 = bp.tile([R, 1], f32, tag="c16")
                nc.vector.tensor_copy(c16[:], ps_t[0:R, 0:1])
                m16 = bp.tile([R, 1], DT.int32, tag="m16")
                nc.vector.tensor_scalar(m16[:], c16[:], float(K), None,
                                        op0=ALU.is_ge)
                lo2 = bp.tile([R, 1], f32, tag="lo")
                nc.vector.select(lo2[:], m16[:], mid[:], lo[:])
                hi2 = bp.tile([R, 1], f32, tag="hi")
                nc.vector.select(hi2[:], m16[:], hi[:], mid[:])
                lo, hi = lo2, hi2
            nc.tensor.matmul(ps_t[:, 1:2], gt_sb[:], lo[:],
                             start=True, stop=True, skip_group_check=True)
            nc.vector.tensor_copy(rbc[:], ps_t[:, 1:2])
            nc.vector.tensor_scalar(sel[:], khot[:], rbc[:], None,
                                    op0=ALU.is_ge)

    if stage < 5:
        nc.sync.dma_start(o_sel.ap(), sel[:])
        return

    # ---- phase 5: epilogue ----
    with tc.tile_pool(name="ph5", bufs=1) as p5, \
         tc.tile_pool(name="ps5", bufs=1, space="PSUM") as ps5:
        lsm = p5.tile([128, F], f32, tag="ph5a")
        nc.vector.tensor_scalar(lsm[:], logits[:], nlse[:], None, op0=ALU.add)
        nc.vector.tensor_tensor(lsm[:], lsm[:], sel[:], ALU.mult)
        lp_ps = ps5.tile([SEG, SEG, 512], f32, tag="lp")
        for j in range(SEG):
            nc.tensor.matmul(lp_ps[:, j, 0:CH], msum_sb[:],
                             lsm[:, j * CH:(j + 1) * CH],
                             start=True, stop=True, skip_group_check=True)
        lp_sb = p5.tile([SEG, F], f32, tag="lpsb")
        nc.vector.tensor_copy(
            lp_sb[:].rearrange("p (j c) -> p j c", c=CH),
            lp_ps[:, :, 0:CH])
        nc.sync.dma_start(o_lp.ap(), lp_sb[:])

        yv = p5.tile([128, F], f32, tag="ph5b")
        nc.vector.tensor_tensor(yv[:], logits[:], sel[:], ALU.mult)
        nc.vector.tensor_scalar(yv[:], yv[:], 0.0, None, op0=ALU.max)
        nc.scalar.activation(yv[:], yv[:], AF.Ln, bias=1.0)
        nc.sync.dma_start(o_y.ap(), yv[:])
        nc.sync.dma_start(o_sel.ap(), sel[:])


def kernel(input_ids, attention_mask, gumbel_u, embed_table, W_vocab, b_vocab):
    input_ids = np.asarray(input_ids)
    attention_mask = np.asarray(attention_mask, np.float32)
    gumbel_u = np.asarray(gumbel_u, np.float32)
    embed_table = np.asarray(embed_table, np.float32)
    W_vocab = np.asarray(W_vocab, np.float32)
    b_vocab = np.asarray(b_vocab, np.float32)

    key = ("nc", K_ITER, STAGE)
    if key not in _cache:
        _cache[key] = build_nc(k_iter=K_ITER, stage=STAGE)
    nc = _cache[key]

    h = embed_table[input_ids.reshape(-1)]          # (128, D)
    Wp = np.zeros((D, VPAD), np.float32)
    Wp[:, :V] = W_vocab
    # (768, VPAD) -> (128, DCH*VPAD): partition p holds rows {c*128+p}
    Wa = np.ascontiguousarray(
        Wp.reshape(DCH, 128, VPAD).transpose(1, 0, 2).reshape(128, -1)
        .astype(ml_dtypes.bfloat16))
    bp = np.full((VPAD,), -1e30, np.float32)
    bp[:V] = b_vocab
    brep = _pack(np.tile(bp, (R, 1)))
    gup = np.full((B * S, VPAD), 0.5, np.float32)
    gup[:, :V] = gumbel_u

    in_maps = []
    for c in range(N_CORES):
        rows = slice(c * R, (c + 1) * R)
        hTa = np.ascontiguousarray(
            h[rows].T.reshape(DCH, 128, R).transpose(1, 0, 2).reshape(128, -1)
            .astype(ml_dtypes.bfloat16))
        in_maps.append({
            "hT": hTa,
            "gu": _pack(gup[rows]),
            "Wv": Wa,
            "bv": brep,
        })

    res = run_bass_kernel_spmd(nc, in_maps, core_ids=list(range(N_CORES)))
    if STAGE < 5:
        return res

    sel = np.concatenate(
        [_unpack(res.results[c]["sel"])[:, :V] for c in range(N_CORES)],
        axis=0).reshape(B, S, V)
    yv = np.concatenate(
        [_unpack(res.results[c]["yv"])[:, :V] for c in range(N_CORES)],
        axis=0).reshape(B, S, V)
    yv = yv * attention_mask.reshape(B, S, 1)
    values = yv.max(axis=1)
    lp = np.stack([res.results[c]["lp"].reshape(-1)[:V] for c in range(N_CORES)])
    logprobs = np.stack([lp[0:4].sum(axis=0), lp[4:8].sum(axis=0)])
    return (values.astype(np.float32), logprobs.astype(np.float32),
            sel.astype(np.float32))
